# revision 15
# baseline (speedup 1.0000x reference)
"""Trainium2 Bass kernel for nn_CompositeLoss (focal + sparsity + concentration).

Strategy (data-parallel over batch, 8 cores x 2 batch = 40 images/core):
  This environment charges a large fixed cost (~35-50us) per *instruction*
  regardless of engine or data size, so the kernel is built around
  instruction-count minimization: 2 chunks x ~17 big ops over [128, 10240]
  tiles + a single tiny matmul finale (~43 instructions total vs ~310 in
  the previous version).

  Layout: partition dim = y-within-half (128), free = (img 20, half 2, x 256)
  per chunk.  Per element: s = 2t-1, w = s*z, q = sigmoid(-w) = 1-pt,
  L = ln(1-q) = ln(pt), q2 = (1-q... wait) Square(q), G = q2*L,
  H = (s<0)*G.  Focal = -(0.25*SG + 0.5*SH)/N.
  Sparsity from bn_stats (sum z, sum z^2), accum SW (sum s*z), abs-reduce.
  Concentration moments: segmented reduces of p=sigmoid(z), s, p*x, s*x,
  p*x*x over the x axis -> [128, stats] tile; one [128,5] stationary matmul
  reduces partitions with [1, y_t, y_t^2, y_b, y_b^2] weights; host (f64)
  combines x/y moments into per-image centroid sums exactly mirroring the
  reference algebra (centered coords; shift-invariant).
"""

import os
import sys
import numpy as np

sys.path.insert(0, "/opt/trn_rl_repo")

B, C, H, W = 16, 20, 256, 256
N_CORES = 8
B_PER_CORE = B // N_CORES            # 2
IMG = B_PER_CORE * C                 # 40 images per core
NCH = 2                              # chunks per rep
FI = IMG // NCH                      # 20 images per chunk
F = FI * 512                         # 10240 free elems per chunk tile
NTOT = float(B * C * H * W)

ALPHA, GAMMA = 0.25, 2.0
SPARSITY_PENALTY = 1.0
FOCAL_W, SPARSITY_W, CONC_W = 1.0, 0.8, 1.5

_PROGRAM_CACHE = {}


def _build_program(reps=1):
    from contextlib import ExitStack
    import concourse.bass as bass
    import concourse.tile as tile
    import concourse.bacc as bacc
    from concourse import mybir

    dt = mybir.dt
    Act = mybir.ActivationFunctionType
    Alu = mybir.AluOpType

    nc = bacc.Bacc("TRN2", target_bir_lowering=False, debug=False,
                   num_devices=N_CORES)

    z_d = nc.dram_tensor("z", [128, NCH, FI, 2, 256], dt.float32,
                         kind="ExternalInput").ap()
    t_d = nc.dram_tensor("t", [128, NCH, FI, 2, 256], dt.float32,
                         kind="ExternalInput").ap()
    xw_d = nc.dram_tensor("xw", [128, 256], dt.float16,
                          kind="ExternalInput").ap()
    wt_d = nc.dram_tensor("wts", [128, 5], dt.float32,
                          kind="ExternalInput").ap()
    moms_d = nc.dram_tensor("moms", [5, NCH, 5, FI, 2], dt.float32,
                            kind="ExternalOutput").ap()
    acc_d = nc.dram_tensor("acc", [128, NCH, 6], dt.float32,
                           kind="ExternalOutput").ap()

    with tile.TileContext(nc) as tc, ExitStack() as ctx:
        const_pool = ctx.enter_context(tc.tile_pool(name="const", bufs=1))
        tz_pool = ctx.enter_context(tc.tile_pool(name="tz", bufs=1))
        big_pool = ctx.enter_context(tc.tile_pool(name="big", bufs=1))
        psum_pool = ctx.enter_context(
            tc.tile_pool(name="psum", bufs=1, space="PSUM"))

        xw = const_pool.tile([128, 256], dt.float16, tag="xw")
        nc.sync.dma_start(xw[:], xw_d[:])
        wt = const_pool.tile([128, 5], dt.float32, tag="wts")
        nc.sync.dma_start(wt[:], wt_d[:])

        # stats: [stat(Rp,Rs,RpX,RsX,RpXX), img, half] per chunk
        S = const_pool.tile([128, NCH, 5, FI, 2], dt.float32, tag="S")
        A = const_pool.tile([128, NCH, 6], dt.float32, tag="A")

        for _ in range(reps):
            for c in range(NCH):
                t32 = tz_pool.tile([128, FI, 2, 256], dt.float32, tag="tz")
                nc.sync.dma_start(t32[:], t_d[:, c])
                z32 = tz_pool.tile([128, FI, 2, 256], dt.float32, tag="tz")
                nc.sync.dma_start(z32[:], z_d[:, c])

                # ps16: p in [:,0], s in [:,1] (adjacent for fused moment ops)
                ps16 = big_pool.tile([128, 2, FI, 2, 256], dt.float16,
                                     tag="ps")
                s_ap = ps16[:, 1]
                p_ap = ps16[:, 0]
                nc.vector.tensor_scalar(s_ap, t32[:], 2.0, -1.0,
                                        Alu.mult, Alu.add)

                # w = s*z (accum -> SW)
                w16 = big_pool.tile([128, FI, 2, 256], dt.float16, tag="w")
                nc.vector.scalar_tensor_tensor(
                    w16[:], s_ap, 0.0, z32[:], Alu.bypass, Alu.mult,
                    accum_out=A[:, c, 0:1])

                # z stats: sum z, sum |z| (DVE), sum z^2 (ACT Square accum)
                nc.vector.tensor_reduce(
                    A[:, c, 4:5], z32[:], mybir.AxisListType.XYZ, Alu.add)
                nc.vector.tensor_reduce(
                    A[:, c, 3:4], z32[:], mybir.AxisListType.XYZ,
                    Alu.add, apply_absolute_value=True)
                z2 = big_pool.tile([128, FI, 2, 256], dt.float16, tag="psx")
                nc.scalar.activation(z2[:], z32[:], Act.Square,
                                     accum_out=A[:, c, 5:6])

                # ACT: p, q (sigmoid set), L (ln set), q2 (square: in-set)
                nc.scalar.activation(p_ap, z32[:], Act.Sigmoid)
                q16 = big_pool.tile([128, FI, 2, 256], dt.float16, tag="q")
                nc.scalar.activation(q16[:], w16[:], Act.Sigmoid, scale=-1.0)
                L16 = big_pool.tile([128, FI, 2, 256], dt.float16, tag="L")
                nc.scalar.activation(L16[:], q16[:], Act.Ln,
                                     scale=-1.0, bias=1.0)
                q2 = big_pool.tile([128, FI, 2, 256], dt.float16, tag="q2")
                nc.scalar.activation(q2[:], q16[:], Act.Square)

                # G = q2*L (accum SG), H = (s<0)*G (accum SH)
                G16 = big_pool.tile([128, FI, 2, 256], dt.float16, tag="w")
                nc.vector.scalar_tensor_tensor(
                    G16[:], q2[:], 0.0, L16[:], Alu.bypass, Alu.mult,
                    accum_out=A[:, c, 1:2])
                H16 = big_pool.tile([128, FI, 2, 256], dt.float16, tag="q2")
                nc.vector.scalar_tensor_tensor(
                    H16[:], s_ap, 0.0, G16[:], Alu.is_lt, Alu.mult,
                    accum_out=A[:, c, 2:3])

                # moments: Rp/Rs, then x-weighted
                nc.vector.tensor_reduce(
                    S[:, c, 0:2], ps16[:], mybir.AxisListType.X, Alu.add)
                xb2 = xw[:].unsqueeze(1).broadcast_to([128, 2 * FI * 2, 256])
                psx = big_pool.tile([128, 2, FI, 2, 256], dt.float16,
                                    tag="psx")
                nc.vector.scalar_tensor_tensor(
                    psx[:].rearrange("p s i h x -> p (s i h) x"),
                    ps16[:].rearrange("p s i h x -> p (s i h) x"),
                    0.0, xb2, Alu.bypass, Alu.mult)
                nc.vector.tensor_reduce(
                    S[:, c, 2:4], psx[:], mybir.AxisListType.X, Alu.add)
                xb1 = xw[:].unsqueeze(1).broadcast_to([128, FI * 2, 256])
                pxx = big_pool.tile([128, FI, 2, 256], dt.float16, tag="w")
                nc.vector.scalar_tensor_tensor(
                    pxx[:].rearrange("p i h x -> p (i h) x"),
                    psx[:, 0].rearrange("p i h x -> p (i h) x"),
                    0.0, xb1, Alu.bypass, Alu.mult)
                nc.vector.tensor_reduce(
                    S[:, c, 4], pxx[:], mybir.AxisListType.X, Alu.add)

            # finale: partition-reduce stats with y-weights
            pm = psum_pool.tile([5, NCH * 5 * FI * 2], dt.float32, tag="pm")
            nc.tensor.matmul(
                pm[:], wt[:],
                S[:].rearrange("p a b c d -> p (a b c d)"),
                start=True, stop=True)
            moms_sb = const_pool.tile([5, NCH * 5 * FI * 2], dt.float32,
                                      tag="momsb")
            nc.vector.tensor_copy(moms_sb[:], pm[:])
            nc.sync.dma_start(
                moms_d[:].rearrange("r a b c d -> r (a b c d)"), moms_sb[:])

        nc.sync.dma_start(acc_d[:], A[:])

    nc.compile()
    return nc


def _get_program(reps=1):
    key = reps
    if key not in _PROGRAM_CACHE:
        _PROGRAM_CACHE[key] = _build_program(reps)
    return _PROGRAM_CACHE[key]


def _host_inputs(pred, target):
    """Build per-core input maps (partition-major layout + coord weights)."""
    p = np.arange(128, dtype=np.float64)
    yh_t = p - 127.5
    yh_b = p + 0.5
    wts = np.stack([np.ones(128), yh_t, yh_t * yh_t, yh_b, yh_b * yh_b],
                   axis=1).astype(np.float32)
    xw = np.broadcast_to((np.arange(256, dtype=np.float64) - 127.5),
                         (128, 256)).astype(np.float16)

    in_maps = []
    for cidx in range(N_CORES):
        b0 = cidx * B_PER_CORE
        z = pred[b0:b0 + B_PER_CORE].reshape(IMG, 2, 128, 256)
        t = target[b0:b0 + B_PER_CORE].reshape(IMG, 2, 128, 256)
        z = np.ascontiguousarray(z.transpose(2, 0, 1, 3)).reshape(
            128, NCH, FI, 2, 256)
        t = np.ascontiguousarray(t.transpose(2, 0, 1, 3)).reshape(
            128, NCH, FI, 2, 256)
        in_maps.append({"z": z, "t": t, "xw": xw, "wts": wts})
    return in_maps


def _finalize(results):
    """Combine per-core outputs into the 4 loss scalars (float64 host math)."""
    SW = SG = SH = SA = Sz = Sz2 = 0.0
    P0l, Pyl, Pyyl, Pxl, Pxxl, S0l, Syl, Sxl = ([] for _ in range(8))
    for r in results:
        acc = r["acc"].astype(np.float64)        # [128, NCH, 6]
        m = r["moms"].astype(np.float64)         # [5, NCH, 5, FI, 2]
        SW += acc[..., 0].sum()
        SG += acc[..., 1].sum()
        SH += acc[..., 2].sum()
        SA += acc[..., 3].sum()
        Sz += acc[..., 4].sum()
        Sz2 += acc[..., 5].sum()
        # m[row, chunk, stat, img, half]; stats: Rp,Rs,RpX,RsX,RpXX
        # rows: 0=plain, 1=y_t, 2=y_t^2, 3=y_b, 4=y_b^2
        P0l.append((m[0, :, 0, :, 0] + m[0, :, 0, :, 1]).ravel())
        Pyl.append((m[1, :, 0, :, 0] + m[3, :, 0, :, 1]).ravel())
        Pyyl.append((m[2, :, 0, :, 0] + m[4, :, 0, :, 1]).ravel())
        S0l.append((m[0, :, 1, :, 0] + m[0, :, 1, :, 1]).ravel())
        Syl.append((m[1, :, 1, :, 0] + m[3, :, 1, :, 1]).ravel())
        Pxl.append((m[0, :, 2, :, 0] + m[0, :, 2, :, 1]).ravel())
        Sxl.append((m[0, :, 3, :, 0] + m[0, :, 3, :, 1]).ravel())
        Pxxl.append((m[0, :, 4, :, 0] + m[0, :, 4, :, 1]).ravel())

    P0 = np.concatenate(P0l)
    Py = np.concatenate(Pyl)
    Pyy = np.concatenate(Pyyl)
    Px = np.concatenate(Pxl)
    Pxx = np.concatenate(Pxxl)
    T0 = (np.concatenate(S0l) + 65536.0) / 2.0
    Ty = np.concatenate(Syl) / 2.0
    Tx = np.concatenate(Sxl) / 2.0

    focal = -(0.25 * SG + 0.5 * SH) / NTOT
    Szt = (SW + Sz) / 2.0
    sparsity = (Sz2 - 2.0 * Szt + T0.sum()) / NTOT \
        + SPARSITY_PENALTY * SA / NTOT

    valid = T0 > 0
    safe = np.where(valid, T0, 1.0)
    cy = Ty / safe
    cx = Tx / safe
    per = (Pyy + Pxx - 2.0 * (cy * Py + cx * Px)
           + (cy * cy + cx * cx) * P0) / float(H * W)
    nv = int(valid.sum())
    conc = (np.where(valid, per, 0.0).sum() / max(nv, 1)) if nv > 0 else 0.0

    total = FOCAL_W * focal + SPARSITY_W * sparsity + CONC_W * conc
    return (np.float32(total), np.float32(focal), np.float32(sparsity),
            np.float32(conc))


def _run(in_maps, reps=1, trace=False):
    nc = _get_program(reps)
    if os.environ.get("BASS_SIM"):
        from concourse.bass_interp import CoreSim
        outs = []
        for im in in_maps:
            sim = CoreSim(nc)
            for name, val in im.items():
                sim.tensor(name)[:] = val
            sim.simulate()
            outs.append({k: np.array(sim.tensor(k))
                         for k in ("moms", "acc")})

        class R:
            results = outs
        return R()
    from concourse.bass_utils import run_bass_kernel_spmd
    last_err = None
    for _ in range(3):
        try:
            return run_bass_kernel_spmd(nc, in_maps, list(range(N_CORES)),
                                        trace=trace)
        except Exception as e:  # transient device errors happen; retry
            last_err = e
    raise last_err


def kernel(pred, target):
    pred = np.ascontiguousarray(pred, dtype=np.float32)
    target = np.ascontiguousarray(target, dtype=np.float32)
    in_maps = _host_inputs(pred, target)
    res = _run(in_maps, reps=int(os.environ.get("KERNEL_REPS", "1")))
    return _finalize(res.results)


# revision 16
# speedup vs baseline: 8.3930x; 8.3930x over previous
"""Trainium2 Bass kernel for nn_CompositeLoss (focal + sparsity + concentration).

This environment charges a large fixed cost (~35-65us) per *instruction*
regardless of engine or data size, so the kernel minimizes instruction
count: ~36 instructions per iteration over big [128, 10240+] tiles.

Data-parallel over batch: 8 cores x 2 batch = 40 images of 256x256 per core.
Layout: partition = y-within-half (128), free = (img, half, x).

Host precomputes s = 2t-1 (fp16, exact) and z (fp16) and ships them; the
device computes, per chunk c of 20 images:
  w = s*z (accum SW), sum z, sum|z| (Abs accum), sum z^2 (Square accum),
  p = sigmoid(z) written next to s.
Then one fused chain over both chunks' w: q = sigmoid(-w) = 1-pt, and per
chunk L = ln(1-q) = ln(pt), q2 = q^2, G = q2*L (accum SG), H = (s<0)*G
(accum SH).  focal = -(0.25*SG + 0.5*SH)/N.
Concentration moments per chunk: segmented reduces over x of [p|s],
[p|s]*x, p*x*x -> stats tile; one [128,5] stationary matmul applies
[1, y_t, y_t^2, y_b, y_b^2] partition weights; scalar accumulators ride
in the same matmul (row 0 = plain partition sum).  Host (f64) combines
moments into per-image centroid sums exactly mirroring the reference
algebra (centered coords; shift-invariant).
"""

import os
import sys
import numpy as np

sys.path.insert(0, "/opt/trn_rl_repo")

B, C, H, W = 16, 20, 256, 256
N_CORES = 8
B_PER_CORE = B // N_CORES            # 2
IMG = B_PER_CORE * C                 # 40 images per core
NCH = 2                              # chunks per rep
FI = IMG // NCH                      # 20 images per chunk
F = FI * 512                         # 10240 free elems per chunk tile
NTOT = float(B * C * H * W)
NS = NCH * 5 * FI * 2                # 400 moment columns
NA = NCH * 6                         # 12 accumulator columns
NSA = NS + NA                        # 412 matmul moving columns

ALPHA, GAMMA = 0.25, 2.0
SPARSITY_PENALTY = 1.0
FOCAL_W, SPARSITY_W, CONC_W = 1.0, 0.8, 1.5

_PROGRAM_CACHE = {}


def _build_program(reps=1):
    from contextlib import ExitStack
    import concourse.bass as bass
    import concourse.tile as tile
    import concourse.bacc as bacc
    from concourse import mybir

    dt = mybir.dt
    Act = mybir.ActivationFunctionType
    Alu = mybir.AluOpType

    nc = bacc.Bacc("TRN2", target_bir_lowering=False, debug=False,
                   num_devices=N_CORES)

    # ps input: [:, c, 1] = s = 2t-1 (fp16), [:, c, 0] = zeros (p's home)
    ps_d = nc.dram_tensor("ps", [128, NCH, 2, FI, 2, 256], dt.float16,
                          kind="ExternalInput").ap()
    z_d = nc.dram_tensor("z", [128, NCH, FI, 2, 256], dt.float16,
                         kind="ExternalInput").ap()
    xw_d = nc.dram_tensor("xw", [128, 256], dt.float16,
                          kind="ExternalInput").ap()
    wt_d = nc.dram_tensor("wts", [128, 5], dt.float32,
                          kind="ExternalInput").ap()
    moms_d = nc.dram_tensor("moms", [5, NSA], dt.float32,
                            kind="ExternalOutput").ap()

    with tile.TileContext(nc) as tc, ExitStack() as ctx:
        const_pool = ctx.enter_context(tc.tile_pool(name="const", bufs=1))
        zp = ctx.enter_context(tc.tile_pool(name="zp", bufs=1))
        big_pool = ctx.enter_context(tc.tile_pool(name="big", bufs=1))
        psum_pool = ctx.enter_context(
            tc.tile_pool(name="psum", bufs=1, space="PSUM"))

        xw = const_pool.tile([128, 256], dt.float16, tag="xw")
        nc.sync.dma_start(xw[:], xw_d[:])
        wt = const_pool.tile([128, 5], dt.float32, tag="wts")
        nc.sync.dma_start(wt[:], wt_d[:])

        # SA: moment stats [.. :NS] as [c, stat, img, half] + accums [NS:]
        SA = const_pool.tile([128, NSA], dt.float32, tag="SA")
        S = SA[:, 0:NS].rearrange("p (c k i h) -> p c k i h",
                                  c=NCH, k=5, i=FI, h=2)

        def acc(c, j):
            return SA[:, NS + c * 6 + j: NS + c * 6 + j + 1]

        for _ in range(reps):
            PS = big_pool.tile([128, NCH, 2, FI, 2, 256], dt.float16,
                               tag="PS")
            Wt = big_pool.tile([128, NCH, FI, 2, 256], dt.float16, tag="W")
            for c in range(NCH):
                nc.sync.dma_start(PS[:, c], ps_d[:, c])
                z16 = zp.tile([128, FI, 2, 256], dt.float16, tag="z")
                nc.sync.dma_start(z16[:], z_d[:, c])

                s_ap = PS[:, c, 1]
                # w = s*z (accum SW)
                nc.vector.scalar_tensor_tensor(
                    Wt[:, c], s_ap, 0.0, z16[:], Alu.bypass, Alu.mult,
                    accum_out=acc(c, 0))
                # sum z (DVE), sum |z| + sum z^2 (ACT, table-free funcs)
                nc.vector.tensor_reduce(
                    acc(c, 4), z16[:], mybir.AxisListType.XYZ, Alu.add)
                nc.scalar.activation(PS[:, c, 0], z16[:], Act.Sigmoid)
                za = big_pool.tile([128, FI, 2, 256], dt.float16, tag="q")
                nc.scalar.activation(za[:], z16[:], Act.Abs,
                                     accum_out=acc(c, 3))
                z2 = big_pool.tile([128, FI, 2, 256], dt.float16, tag="q")
                nc.scalar.activation(z2[:], z16[:], Act.Square,
                                     accum_out=acc(c, 5))

            # fused focal chain: q over both chunks, then per-chunk L/G/H
            Q = big_pool.tile([128, NCH, FI, 2, 256], dt.float16, tag="q")
            nc.scalar.activation(Q[:], Wt[:], Act.Sigmoid, scale=-1.0)
            for c in range(NCH):
                L16 = big_pool.tile([128, FI, 2, 256], dt.float16, tag="L")
                nc.scalar.activation(L16[:], Q[:, c], Act.Ln,
                                     scale=-1.0, bias=1.0)
                q2 = big_pool.tile([128, FI, 2, 256], dt.float16, tag="W")
                nc.scalar.activation(q2[:], Q[:, c], Act.Square)
                G16 = zp.tile([128, FI, 2, 256], dt.float16, tag="z")
                nc.vector.scalar_tensor_tensor(
                    G16[:], q2[:], 0.0, L16[:], Alu.bypass, Alu.mult,
                    accum_out=acc(c, 1))
                H16 = big_pool.tile([128, FI, 2, 256], dt.float16, tag="W")
                nc.vector.scalar_tensor_tensor(
                    H16[:], PS[:, c, 1], 0.0, G16[:], Alu.is_lt, Alu.mult,
                    accum_out=acc(c, 2))

            # concentration moments per chunk
            for c in range(NCH):
                nc.vector.tensor_reduce(
                    S[:, c, 0:2], PS[:, c], mybir.AxisListType.X, Alu.add)
                xb2 = xw[:].unsqueeze(1).broadcast_to([128, 2 * FI * 2, 256])
                psx = big_pool.tile([128, 2, FI, 2, 256], dt.float16,
                                    tag="q")
                nc.vector.scalar_tensor_tensor(
                    psx[:].rearrange("p s i h x -> p (s i h) x"),
                    PS[:, c].rearrange("p s i h x -> p (s i h) x"),
                    0.0, xb2, Alu.bypass, Alu.mult)
                nc.vector.tensor_reduce(
                    S[:, c, 2:4], psx[:], mybir.AxisListType.X, Alu.add)
                xb1 = xw[:].unsqueeze(1).broadcast_to([128, FI * 2, 256])
                pxx = big_pool.tile([128, FI, 2, 256], dt.float16, tag="W")
                nc.vector.scalar_tensor_tensor(
                    pxx[:].rearrange("p i h x -> p (i h) x"),
                    psx[:, 0].rearrange("p i h x -> p (i h) x"),
                    0.0, xb1, Alu.bypass, Alu.mult)
                nc.vector.tensor_reduce(
                    S[:, c, 4], pxx[:], mybir.AxisListType.X, Alu.add)

            # finale: partition-reduce stats + accumulators with y-weights
            pm = psum_pool.tile([5, NSA], dt.float32, tag="pm")
            nc.tensor.matmul(pm[:], wt[:], SA[:], start=True, stop=True)
            moms_sb = const_pool.tile([5, NSA], dt.float32, tag="momsb")
            nc.vector.tensor_copy(moms_sb[:], pm[:])
            nc.sync.dma_start(moms_d[:], moms_sb[:])

    nc.compile()
    return nc


def _get_program(reps=1):
    key = reps
    if key not in _PROGRAM_CACHE:
        _PROGRAM_CACHE[key] = _build_program(reps)
    return _PROGRAM_CACHE[key]


def _host_inputs(pred, target):
    """Build per-core input maps (partition-major layout + coord weights)."""
    p = np.arange(128, dtype=np.float64)
    yh_t = p - 127.5
    yh_b = p + 0.5
    wts = np.stack([np.ones(128), yh_t, yh_t * yh_t, yh_b, yh_b * yh_b],
                   axis=1).astype(np.float32)
    xw = np.broadcast_to((np.arange(256, dtype=np.float64) - 127.5),
                         (128, 256)).astype(np.float16)

    in_maps = []
    for cidx in range(N_CORES):
        b0 = cidx * B_PER_CORE
        z = pred[b0:b0 + B_PER_CORE].reshape(IMG, 2, 128, 256)
        t = target[b0:b0 + B_PER_CORE].reshape(IMG, 2, 128, 256)
        z = np.ascontiguousarray(z.transpose(2, 0, 1, 3)).reshape(
            128, NCH, FI, 2, 256).astype(np.float16)
        s = (2.0 * t - 1.0).astype(np.float16)
        ps = np.zeros((128, NCH, 2, FI, 2, 256), np.float16)
        ps[:, :, 1] = np.ascontiguousarray(s.transpose(2, 0, 1, 3)).reshape(
            128, NCH, FI, 2, 256)
        in_maps.append({"ps": ps, "z": z, "xw": xw, "wts": wts})
    return in_maps


def _finalize(results):
    """Combine per-core outputs into the 4 loss scalars (float64 host math)."""
    SW = SG = SH = SA_ = Sz = Sz2 = 0.0
    P0l, Pyl, Pyyl, Pxl, Pxxl, S0l, Syl, Sxl = ([] for _ in range(8))
    for r in results:
        mfull = r["moms"].astype(np.float64)     # [5, NSA]
        a = mfull[0, NS:].reshape(NCH, 6)        # row0 = plain partition sum
        SW += a[:, 0].sum()
        SG += a[:, 1].sum()
        SH += a[:, 2].sum()
        SA_ += a[:, 3].sum()
        Sz += a[:, 4].sum()
        Sz2 += a[:, 5].sum()
        m = mfull[:, :NS].reshape(5, NCH, 5, FI, 2)
        # m[row, chunk, stat, img, half]; stats: Rp,Rs,RpX,RsX,RpXX
        # rows: 0=plain, 1=y_t, 2=y_t^2, 3=y_b, 4=y_b^2
        P0l.append((m[0, :, 0, :, 0] + m[0, :, 0, :, 1]).ravel())
        Pyl.append((m[1, :, 0, :, 0] + m[3, :, 0, :, 1]).ravel())
        Pyyl.append((m[2, :, 0, :, 0] + m[4, :, 0, :, 1]).ravel())
        S0l.append((m[0, :, 1, :, 0] + m[0, :, 1, :, 1]).ravel())
        Syl.append((m[1, :, 1, :, 0] + m[3, :, 1, :, 1]).ravel())
        Pxl.append((m[0, :, 2, :, 0] + m[0, :, 2, :, 1]).ravel())
        Sxl.append((m[0, :, 3, :, 0] + m[0, :, 3, :, 1]).ravel())
        Pxxl.append((m[0, :, 4, :, 0] + m[0, :, 4, :, 1]).ravel())

    P0 = np.concatenate(P0l)
    Py = np.concatenate(Pyl)
    Pyy = np.concatenate(Pyyl)
    Px = np.concatenate(Pxl)
    Pxx = np.concatenate(Pxxl)
    T0 = (np.concatenate(S0l) + 65536.0) / 2.0
    Ty = np.concatenate(Syl) / 2.0
    Tx = np.concatenate(Sxl) / 2.0

    focal = -(0.25 * SG + 0.5 * SH) / NTOT
    Szt = (SW + Sz) / 2.0
    sparsity = (Sz2 - 2.0 * Szt + T0.sum()) / NTOT \
        + SPARSITY_PENALTY * SA_ / NTOT

    valid = T0 > 0
    safe = np.where(valid, T0, 1.0)
    cy = Ty / safe
    cx = Tx / safe
    per = (Pyy + Pxx - 2.0 * (cy * Py + cx * Px)
           + (cy * cy + cx * cx) * P0) / float(H * W)
    nv = int(valid.sum())
    conc = (np.where(valid, per, 0.0).sum() / max(nv, 1)) if nv > 0 else 0.0

    total = FOCAL_W * focal + SPARSITY_W * sparsity + CONC_W * conc
    return (np.float32(total), np.float32(focal), np.float32(sparsity),
            np.float32(conc))


def _run(in_maps, reps=1, trace=False):
    nc = _get_program(reps)
    if os.environ.get("BASS_SIM"):
        from concourse.bass_interp import CoreSim
        outs = []
        for im in in_maps:
            sim = CoreSim(nc)
            for name, val in im.items():
                sim.tensor(name)[:] = val
            sim.simulate()
            outs.append({"moms": np.array(sim.tensor("moms"))})

        class R:
            results = outs
        return R()
    from concourse.bass_utils import run_bass_kernel_spmd
    last_err = None
    for _ in range(3):
        try:
            return run_bass_kernel_spmd(nc, in_maps, list(range(N_CORES)),
                                        trace=trace)
        except Exception as e:  # transient device errors happen; retry
            last_err = e
    raise last_err


def kernel(pred, target):
    pred = np.ascontiguousarray(pred, dtype=np.float32)
    target = np.ascontiguousarray(target, dtype=np.float32)
    in_maps = _host_inputs(pred, target)
    res = _run(in_maps, reps=int(os.environ.get("KERNEL_REPS", "1")))
    return _finalize(res.results)
